# revision 14
# baseline (speedup 1.0000x reference)
"""Binary-weight 3x3 conv (sign(W)), NCHW, stride 1, pad 1, on 8 trn2 cores.

Full inputs:  x [32,128,56,56] f32, W [256,128,3,3] f32
Full output:  out [32,256,56,56] f32

Strategy: data-parallel over batch (4 images/core). Per core, implicit GEMM:
for each of the 9 kernel offsets, a [K=C=128, M=O=128] x [K=128, N=448]
bf16 matmul accumulating into PSUM (fp32). Images arrive host-padded with a
1-pixel zero halo so every offset is a clean shifted window.

Perf structure:
- block-outer loop: 9 accumulating matmuls per 8-row output block, then the
  PSUM bank is drained (VectorE/ScalarE alternating) while the next block's
  matmuls run. Only 4 PSUM banks needed.
- each image is loaded as four independent row-chunks (18/18/18/10 rows) and
  the weights as two 128-output halves, so the first matmul gates on ~560KB
  of DMA instead of the whole input;
- a short burst of warmup matmuls on a zeroed scratch tile runs during the
  input DMA to lift the PE out of its cold clock-gate (HAM K=4/8).

Weights are binarized (+-1, exactly representable in bf16) on host; x is cast
to bf16 on host (halves DMA traffic; rounding error ~1e-3 relative).
"""

import numpy as np
import ml_dtypes

import concourse.bacc as bacc
import concourse.mybir as mybir
from concourse.tile import TileContext
from concourse.bass_utils import run_bass_kernel_spmd

N_CORES = 8
IMGS = 4          # images per core (32 / 8)
C = 128           # input channels  = contraction dim = partitions
O = 256           # output channels
H = WD = 56
HP = WP = 58      # padded spatial
KH = KW = 3
RB = 8            # output rows per matmul block
NBLK = H // RB    # 7 blocks per image
P = 128
N_WARM = 6        # warmup matmuls

# row-chunks of the padded image; BLK_CHUNK maps block -> chunk
CHUNKS = [(0, 10), (8, 18), (24, 18), (40, 18)]  # (start_row, n_rows)
BLK_CHUNK = [0, 1, 1, 2, 2, 3, 3]

BF16 = mybir.dt.bfloat16
F32 = mybir.dt.float32


def build_nc():
    nc = bacc.Bacc(None, target_bir_lowering=False)
    x = nc.dram_tensor("x", [IMGS, C, HP, WP], BF16, kind="ExternalInput")
    wb = nc.dram_tensor("wb", [C, 2, KH, KW, P], BF16, kind="ExternalInput")
    out = nc.dram_tensor("out", [IMGS, O, H, WD], F32, kind="ExternalOutput")

    with TileContext(nc) as tc:
        with (
            tc.tile_pool(name="wpool", bufs=1) as wpool,
            tc.tile_pool(name="xpool", bufs=1) as xpool,
            tc.tile_pool(name="opool", bufs=8) as opool,
            tc.tile_pool(name="psum", bufs=4, space="PSUM") as psum_pool,
        ):
            wt = wpool.tile([P, 2, KH, KW, P], BF16, name="wt")
            wsc = wpool.tile([P, 512], BF16, name="wsc")
            nc.gpsimd.memset(wsc[:], 0.0)

            # chunk tiles: [P, IMGS, nrows, WP] per chunk index
            xts = [
                xpool.tile([P, IMGS, nr, WP], BF16, name=f"xc{ci}")
                for ci, (_, nr) in enumerate(CHUNKS)
            ]

            # DMA dispatch order = urgency order: first x chunk, then the
            # first-needed weight rows, interleaved with the rest of img0
            nc.sync.dma_start(out=xts[0][:, 0], in_=x[0, :, 0:CHUNKS[0][1]])
            nc.sync.dma_start(out=wt[:, 0, 0], in_=wb[:, 0, 0])
            nc.sync.dma_start(out=wt[:, 0, 1], in_=wb[:, 0, 1])
            nc.sync.dma_start(out=wt[:, 0, 2], in_=wb[:, 0, 2])
            nc.sync.dma_start(out=xts[1][:, 0],
                              in_=x[0, :, CHUNKS[1][0]:CHUNKS[1][0] + CHUNKS[1][1]])
            for ci, (r0, nr) in list(enumerate(CHUNKS))[2:]:
                nc.sync.dma_start(out=xts[ci][:, 0], in_=x[0, :, r0:r0 + nr])
            for kh in range(KH):
                nc.sync.dma_start(out=wt[:, 1, kh], in_=wb[:, 1, kh])
            for img in range(1, IMGS):
                for ci, (r0, nr) in enumerate(CHUNKS):
                    nc.sync.dma_start(out=xts[ci][:, img], in_=x[img, :, r0:r0 + nr])

            # warmup: PE activity during the input DMA so HAM reaches K=8/8
            warm = psum_pool.tile([P, RB, WD], F32, name="warm", tag="pst")
            for _ in range(N_WARM):
                nc.tensor.matmul(
                    warm[:], lhsT=wsc[:, :P], rhs=wsc[:, :RB * WD],
                    start=True, stop=True,
                )

            for img in range(IMGS):
                for half in range(2):
                    for blk in range(NBLK):
                        ci = BLK_CHUNK[blk]
                        cr0 = CHUNKS[ci][0]
                        pst = psum_pool.tile([P, RB, WD], F32, name="pst", tag="pst")
                        for ki in range(KH * KW):
                            kh, kw = divmod(ki, KW)
                            r0 = blk * RB + kh - cr0
                            nc.tensor.matmul(
                                pst[:],
                                lhsT=wt[:, half, kh, kw, :],
                                rhs=xts[ci][:, img, r0:r0 + RB, kw:kw + WD],
                                start=(ki == 0),
                                stop=(ki == KH * KW - 1),
                            )
                        ot = opool.tile([P, RB, WD], F32, name="ot", tag="ot")
                        if blk % 2 == 0:
                            nc.vector.tensor_copy(ot[:], pst[:])
                        else:
                            nc.scalar.copy(out=ot[:], in_=pst[:])
                        nc.sync.dma_start(
                            out=out[img, half * P:(half + 1) * P,
                                    blk * RB:(blk + 1) * RB, :],
                            in_=ot[:],
                        )
    nc.compile()
    return nc


_NC_CACHE = None


def _get_nc():
    global _NC_CACHE
    if _NC_CACHE is None:
        _NC_CACHE = build_nc()
    return _NC_CACHE


def prep_inputs(x: np.ndarray, W: np.ndarray):
    """Host-side prep: binarize weights, cast to bf16, pad, shard over cores."""
    xb = np.asarray(x).astype(ml_dtypes.bfloat16)
    xp = np.zeros((xb.shape[0], C, HP, WP), dtype=ml_dtypes.bfloat16)
    xp[:, :, 1:H + 1, 1:WD + 1] = xb
    # [O,C,3,3] -> [C, 2, KH, KW, 128]  (output-half major for split DMA)
    wsign = np.sign(np.asarray(W)).astype(ml_dtypes.bfloat16)
    wbt = np.ascontiguousarray(
        wsign.reshape(2, P, C, KH, KW).transpose(2, 0, 3, 4, 1)
    )
    xs = xp.reshape(N_CORES, IMGS, C, HP, WP)
    return [{"x": np.ascontiguousarray(xs[c]), "wb": wbt} for c in range(N_CORES)]


def kernel(x: np.ndarray, W: np.ndarray) -> np.ndarray:
    nc = _get_nc()
    in_maps = prep_inputs(x, W)
    res = run_bass_kernel_spmd(nc, in_maps, core_ids=list(range(N_CORES)))
    outs = [res.results[c]["out"] for c in range(N_CORES)]
    return np.concatenate(outs, axis=0).astype(np.float32)
